# revision 16
# baseline (speedup 1.0000x reference)
"""BertSelfAttention on 8 Trainium2 NeuronCores (Bass/Tile, SPMD). V3.

Problem: B=2, S=2048, D=1024, H=16 heads, head_dim=64.
Sharding: core c handles batch b = c//4 and heads [4*(c%4), 4*(c%4)+4)
(data parallel on B x tensor parallel on heads). Scores stay core-local.

V3 changes vs V2 (245.5us):
  - 2-step software pipeline (smm bufs=3, acc bufs=2): ctx(s) issues two
    steps after scores(s), hiding the full exp latency (~1.3us) that V2's
    1-step lookahead missed (340ns stall + pipeline refill per step).
  - 1/4 of the exp work offloaded from ACT to DVE via two runtime-registered
    custom DVE ops: EXP_P4_ANT (deg-4 poly ~ exp(s/128), rel err 2e-4) and
    POW16_ANT (four chained squarings -> exp(s/8)). Scores are pre-scaled by
    1/128 in the qT evacuation (free: two-op tensor_scalar); ACT exps use
    scale=16. Lookahead is 3 steps (the scores PSUM tile is released by the
    first DVE instruction, so smm bufs=3 still suffices).
  - Block evacuations split ACT/DVE and acc pool shrunk to 2 bufs.
  - qk(j1) filler split into q/k halves for finer interleave.

Math notes (exact transformations vs the reference):
  - bk dropped: scores[i,j] += q_i . bk is constant in j -> softmax invariant.
  - bv added host-side: softmax rows sum to 1 -> probs @ (1 x bv) = bv.
  - host divides ctxT rows 0..63 by row 64 (softmax denominator) + transposes.
  - additive mask folded multiplicatively via v_ext (65th col = exp(mask)).
"""

import numpy as np
import ml_dtypes
from contextlib import ExitStack
BF = ml_dtypes.bfloat16

B, S, D, H = 2, 2048, 1024, 16
HD = 64
N_CORES = 8
HPC = 4            # heads per core
CW = HPC * HD      # 256 output cols per core
KI = D // 128      # 8 contraction chunks
NSC = S // 128     # 16 k chunks of 128
NQH = 2            # q halves per head
NWARM = 10         # PE warm-up matmuls
USE_DVE_EXP = True

# deg-4 minimax for exp(w), w in [-0.58, 0.58], constrained a0=a1=1:
# exp(w) ~= (1+w) + w^2*((c2 + c3*w) + c4*w^2); rel err 2.0e-4, x16 -> 3.2e-3
EXP_C2 = 0.5007046190058639
EXP_C3 = 0.16849555077816972
EXP_C4 = 0.03865395001612302

_NC_CACHE = []
_DVE_OPS = []


def _register_dve_exp():
    """Register EXP_P4_ANT + POW16_ANT via the documented dve_ops extension
    point (runtime append; the RO install prevents editing dve_ops.py)."""
    if _DVE_OPS:
        return _DVE_OPS
    from concourse import dve_ops
    from concourse.dve_spec import Spec, Src0, One, C0, C1, C2, lower
    from concourse.dve_uop import DveOpSpec
    from concourse.dve_table_gen import dve_ver_for

    ver = dve_ver_for("TRN2")

    w = Src0
    w2 = w * w
    poly = (One + w) + ((C0 + C1 * w) + C2 * w2) * w2
    x = Src0
    x2 = x * x
    x4 = x2 * x2
    x8 = x4 * x4
    specs = [
        ("EXP_P4_ANT", Spec(body=poly)),
        ("POW16_ANT", Spec(body=x8 * x8)),
    ]
    ops = []
    for name, spec in specs:
        existing = [o for o in dve_ops.OPS if o.name == name]
        if existing:
            ops.append(existing[0])
            continue
        row = dve_ops._CUSTOM_DVE_ROW_BASE + len(dve_ops.OPS)
        sha = DveOpSpec(name=name, opcode=row, uops=lower(spec, ver=ver),
                        rd1_en=False).sha(ver)
        op = dve_ops.DveOp(name, spec, subdim=False, uops_sha={ver: sha})
        dve_ops.OPS.append(op)
        dve_ops._SUB_OPCODE_FOR_NAME[name] = row
        ops.append(op)
    _DVE_OPS.extend(ops)
    return ops


def _build_nc():
    import concourse.bacc as bacc
    import concourse.mybir as mybir
    import concourse.tile as tile
    from concourse import masks

    exp_p4, pow16 = _register_dve_exp()

    F32 = mybir.dt.float32
    F32R = mybir.dt.float32r
    BF16 = mybir.dt.bfloat16
    AF = mybir.ActivationFunctionType
    ALU = mybir.AluOpType

    nc = bacc.Bacc("TRN2", target_bir_lowering=False, debug=False)

    hsT_d = nc.dram_tensor("hsT", [D, S], BF16, kind="ExternalInput")
    wq_d = nc.dram_tensor("wq", [D, CW], BF16, kind="ExternalInput")
    wk_d = nc.dram_tensor("wk", [D, CW], BF16, kind="ExternalInput")
    wv_d = nc.dram_tensor("wv", [D, CW], BF16, kind="ExternalInput")
    bq_d = nc.dram_tensor("bq", [CW], F32, kind="ExternalInput")
    mask_d = nc.dram_tensor("mask", [S], F32, kind="ExternalInput")
    # ctxT + denominator row per head: rows 65h..65h+63 = ctx.T, 65h+64 = den
    out_d = nc.dram_tensor("out", [HPC * (HD + 1), S], F32, kind="ExternalOutput")

    hsT_r = hsT_d.rearrange("(ki p) s -> p ki s", p=128)
    wq_r = wq_d.rearrange("(ki p) n -> p ki n", p=128)
    wk_r = wk_d.rearrange("(ki p) n -> p ki n", p=128)
    wv_r = wv_d.rearrange("(ki p) n -> p ki n", p=128)
    bq_r = bq_d.rearrange("(j p) -> p j", p=128)
    mask_r = mask_d.rearrange("(sc p) -> p sc", p=128)

    with tile.TileContext(nc) as tc, ExitStack() as ctx:
        const = ctx.enter_context(tc.tile_pool(name="const", bufs=1))
        load = ctx.enter_context(tc.tile_pool(name="load", bufs=1))
        acts = ctx.enter_context(tc.tile_pool(name="acts", bufs=1))
        ptp = ctx.enter_context(tc.tile_pool(name="ptp", bufs=6))
        outp = ctx.enter_context(tc.tile_pool(name="outp", bufs=4))
        # PSUM: smm "mm" 3x[128,1024] = 6 banks; ctxa "acc" 2x[128,512] = 2.
        smm = ctx.enter_context(tc.tile_pool(name="smm", bufs=3, space="PSUM"))
        ctxa = ctx.enter_context(tc.tile_pool(name="ctxa", bufs=2, space="PSUM"))

        # ---- constants ----
        warmsrc = const.tile([128, 128], F32)
        nc.vector.memset(warmsrc[:], 0.0)
        bq_t = const.tile([128, 2], F32)
        nc.sync.dma_start(bq_t[:], bq_r)
        maskr = const.tile([128, 16], F32)
        nc.sync.dma_start(maskr[:], mask_r)
        emt = const.tile([128, 16], F32)
        nc.scalar.activation(emt[:], maskr[:], AF.Exp)

        # ---- PE warm-up: fp32 identity matmuls while DMAs land (HAM) ----
        warm = smm.tile([128, 1024], F32, tag="mm", name="warm")
        for i in range(NWARM):
            nc.tensor.matmul(warm[:, 0:128], warmsrc[:], warmsrc[:],
                             start=True, stop=True)

        # ---- loads: critical-first on HWDGE(sync); bulk on SWDGE(gpsimd) ----
        hsT_t = load.tile([128, KI, S], BF16)
        wq_t = load.tile([128, KI, CW], BF16)
        wk_t = load.tile([128, KI, CW], BF16)
        wv_t = load.tile([128, KI, CW], BF16)
        for ki in range(KI):
            nc.sync.dma_start(wq_t[:, ki, 0:128], wq_r[:, ki, 0:128])
            nc.sync.dma_start(hsT_t[:, ki, 0:512], hsT_r[:, ki, 0:512])
        for ki in range(KI):
            nc.sync.dma_start(wk_t[:, ki, 0:128], wk_r[:, ki, 0:128])
        for n in range(1, 4):
            for ki in range(KI):
                nc.sync.dma_start(hsT_t[:, ki, 512 * n:512 * (n + 1)],
                                  hsT_r[:, ki, 512 * n:512 * (n + 1)])
        nc.vector.tensor_scalar_add(wq_t[0:1, :, 128:129],
                                    hsT_t[0:1, 0:KI, 0:1], 0.0)
        nc.vector.tensor_scalar_add(wk_t[0:1, :, 128:129],
                                    hsT_t[0:1, 0:KI, 0:1], 0.0)
        nc.vector.tensor_scalar_add(wv_t[0:1, :, 0:1],
                                    hsT_t[0:1, 0:KI, 0:1], 0.0)
        for ki in range(KI):
            nc.gpsimd.dma_start(wq_t[:, ki, 128:256], wq_r[:, ki, 128:256])
        for ki in range(KI):
            nc.gpsimd.dma_start(wk_t[:, ki, 128:256], wk_r[:, ki, 128:256])
        for ki in range(KI):
            nc.gpsimd.dma_start(wv_t[:, ki, :], wv_r[:, ki, :])

        # ---- persistent activations ----
        qT_t = acts.tile([128, 2, S], BF16)
        kT_t = acts.tile([128, 2, S], BF16)
        v_ext = acts.tile([128, NSC, HPC, HD + 1], BF16)
        for h in range(HPC):
            nc.vector.tensor_copy(v_ext[:, :, h, HD], emt[:, :])

        # ---- Q^T (pre-scaled by 1/128) and K^T (zero-padded) ----
        def emit_qk_q(j, n, ps):
            for ki in range(KI):
                nc.tensor.matmul(
                    ps[:, 0:512], wq_t[:, ki, 128 * j:128 * (j + 1)],
                    hsT_t[:, ki, 512 * n:512 * (n + 1)],
                    start=(ki == 0), stop=(ki == KI - 1))
            nc.vector.tensor_scalar(
                qT_t[:, j, 512 * n:512 * (n + 1)], ps[:, 0:512],
                bq_t[:, j:j + 1], 1.0 / 128.0, ALU.add, ALU.mult)

        def emit_qk_k(j, n, ps):
            for ki in range(KI):
                nc.tensor.matmul(
                    ps[:, 512:1024], wk_t[:, ki, 128 * j:128 * (j + 1)],
                    hsT_t[:, ki, 512 * n:512 * (n + 1)],
                    start=(ki == 0), stop=(ki == KI - 1))
            nc.vector.tensor_copy(kT_t[:, j, 512 * n:512 * (n + 1)],
                                  ps[:, 512:1024])

        def emit_qk_n(j, n):
            ps = smm.tile([128, 1024], F32, tag="mm", name=f"qk{j}_{n}")
            emit_qk_q(j, n, ps)
            emit_qk_k(j, n, ps)

        # ---- V chunk sc: [128, 256] -> v_ext[:, sc, :, 0:64] * exp(mask) ----
        def emit_v(sc):
            ps4 = smm.tile([128, 16, 64], F32, tag="mm", name=f"v{sc}")
            for ki in range(KI):
                nc.tensor.matmul(
                    ps4[:, 0:4, :], hsT_t[:, ki, 128 * sc:128 * (sc + 1)],
                    wv_t[:, ki, :], start=(ki == 0), stop=(ki == KI - 1))
            nc.vector.tensor_scalar_mul(
                v_ext[:, sc, :, 0:HD], ps4[:, 0:4, :], emt[:, sc:sc + 1])

        # ---- attention pipeline ----
        # step s = (h, qh, kc). DVE handles exp when kc % 4 == 1 (never the
        # start/stop kc of an accumulation block), ACT otherwise.
        steps = [(j, qc, kc)
                 for j in range(2) for qc in range(4) for kc in range(NSC)]
        nsteps = len(steps)

        def is_dve(s):
            return USE_DVE_EXP and steps[s][2] in (1, 5, 9, 13)

        # filler PE work interleaved into early steps (keyed by sc-emission idx)
        filler = {}
        for kc in range(NSC):          # V chunks during block 0, even slots
            filler.setdefault(kc + (kc % 2), []).append(("v", kc))
        for n in range(4):             # qk j=1 halves during (h0, qh1)
            filler.setdefault(NSC + 4 * n, []).append(("qkq", n))
            filler.setdefault(NSC + 4 * n + 2, []).append(("qkk", n))

        acc_tiles = {}
        pt_tiles = {}
        qk1_ps = {}

        def emit_scores(s):
            j, qc, kc = steps[s]
            ps = smm.tile([128, 1024], F32, tag="mm", name=f"sc{s}")
            qs = slice(512 * qc, 512 * (qc + 1))
            for par in range(2):
                pr = slice(64 * par, 64 * (par + 1))
                nc.tensor.matmul(
                    ps[:, 512 * par:512 * (par + 1)],
                    kT_t[pr, j, 128 * kc:128 * (kc + 1)],
                    qT_t[pr, j, qs],
                    start=True, stop=True, tile_position=(64 * par, 0))
            pt = ptp.tile([128, 1024], BF16, tag="pt", name=f"pt{s}")
            if is_dve(s):
                tmp = ptp.tile([128, 1024], F32, tag="ptm", bufs=2,
                               name=f"tm{s}")
                nc.vector._custom_dve(exp_p4, out=tmp[:], in0=ps[:],
                                      s0=EXP_C2, s1=EXP_C3, imm2=EXP_C4)
                nc.vector._custom_dve(pow16, out=pt[:], in0=tmp[:])
            else:
                nc.scalar.activation(pt[:], ps[:], AF.Exp, scale=16.0)
            pt_tiles[s] = pt

        def emit_ctx(s):
            j, qc, kc = steps[s]
            if kc == 0:
                acc_tiles[s] = [
                    ctxa.tile([128, 512], F32, tag="acc", name=f"acc{s}_{p}")
                    for p in range(2)]
            accs = acc_tiles[s - kc]
            pt = pt_tiles.pop(s)
            for par in range(2):
                nc.tensor.matmul(
                    accs[par][0:65, :], v_ext[:, kc, 2 * j + par, :],
                    pt[:, 512 * par:512 * (par + 1)],
                    start=(kc == 0), stop=(kc == NSC - 1))
            if kc == NSC - 1:
                for par in range(2):
                    h = 2 * j + par
                    cts = outp.tile([65, 512], F32, tag="cts", name=f"c{s}_{par}")
                    nc.scalar.copy(cts[:, 0:256], accs[par][0:65, 0:256])
                    nc.vector.tensor_copy(cts[:, 256:512],
                                          accs[par][0:65, 256:512])
                    nc.sync.dma_start(
                        out_d[65 * h:65 * (h + 1),
                              512 * qc:512 * (qc + 1)], cts[:])
                del acc_tiles[s - kc]

        for n in range(4):
            emit_qk_n(0, n)
        for i in range(nsteps):
            for kind, arg in filler.get(i, []):
                if kind == "v":
                    emit_v(arg)
                elif kind == "qkq":
                    ps = smm.tile([128, 1024], F32, tag="mm", name=f"q1_{arg}")
                    qk1_ps[arg] = ps
                    emit_qk_q(1, arg, ps)
                else:
                    emit_qk_k(1, arg, qk1_ps.pop(arg))
            emit_scores(i)
            if i % 2 == 1 and i >= 5:
                emit_ctx(i - 5)
                emit_ctx(i - 4)
        for s in range(nsteps - 4, nsteps):
            emit_ctx(s)

    nc.finalize()
    return nc


def _get_nc():
    if not _NC_CACHE:
        _NC_CACHE.append(_build_nc())
    return _NC_CACHE[0]


def _shard_inputs(hidden_states, attention_mask, Wq, bq, Wk, Wv):
    hsT = [np.ascontiguousarray(hidden_states[b].T) for b in range(B)]
    in_maps = []
    for c in range(N_CORES):
        b, g = divmod(c, N_CORES // B)
        cs = slice(CW * g, CW * (g + 1))
        in_maps.append({
            "hsT": hsT[b].astype(BF),
            "wq": np.ascontiguousarray(Wq[:, cs]).astype(BF),
            "wk": np.ascontiguousarray(Wk[:, cs]).astype(BF),
            "wv": np.ascontiguousarray(Wv[:, cs]).astype(BF),
            "bq": np.ascontiguousarray(bq[cs]),
            "mask": np.ascontiguousarray(attention_mask[b, 0, 0, :]),
        })
    return in_maps


def kernel(hidden_states, attention_mask, Wq, bq, Wk, bk, Wv, bv):
    from concourse.bass_utils import run_bass_kernel_spmd

    hidden_states = np.asarray(hidden_states, dtype=np.float32)
    attention_mask = np.asarray(attention_mask, dtype=np.float32)
    Wq = np.asarray(Wq, dtype=np.float32)
    Wk = np.asarray(Wk, dtype=np.float32)
    Wv = np.asarray(Wv, dtype=np.float32)
    bq = np.asarray(bq, dtype=np.float32)
    bv = np.asarray(bv, dtype=np.float32)

    in_maps = _shard_inputs(hidden_states, attention_mask, Wq, bq, Wk, Wv)
    res = run_bass_kernel_spmd(_get_nc(), in_maps, core_ids=list(range(N_CORES)))

    out = np.empty((B, S, D), dtype=np.float32)
    for c in range(N_CORES):
        b, g = divmod(c, N_CORES // B)
        o = res.results[c]["out"].astype(np.float64)  # [260, 2048]
        for h in range(HPC):
            ctx = o[65 * h:65 * h + 64, :] / o[65 * h + 64, :]
            out[b, :, CW * g + HD * h:CW * g + HD * (h + 1)] = ctx.T
    out += bv  # exact: probs rows sum to 1
    return out


# revision 17
# speedup vs baseline: 1.0114x; 1.0114x over previous
"""BertSelfAttention on 8 Trainium2 NeuronCores (Bass/Tile, SPMD). V3.

Problem: B=2, S=2048, D=1024, H=16 heads, head_dim=64.
Sharding: core c handles batch b = c//4 and heads [4*(c%4), 4*(c%4)+4)
(data parallel on B x tensor parallel on heads). Scores stay core-local.

V3 changes vs V2 (245.5us):
  - 2-step software pipeline (smm bufs=3, acc bufs=2): ctx(s) issues two
    steps after scores(s), hiding the full exp latency (~1.3us) that V2's
    1-step lookahead missed (340ns stall + pipeline refill per step).
  - 1/4 of the exp work offloaded from ACT to DVE via two runtime-registered
    custom DVE ops: EXP_P4_ANT (deg-4 poly ~ exp(s/128), rel err 2e-4) and
    POW16_ANT (four chained squarings -> exp(s/8)). Scores are pre-scaled by
    1/128 in the qT evacuation (free: two-op tensor_scalar); ACT exps use
    scale=16. Lookahead is 3 steps (the scores PSUM tile is released by the
    first DVE instruction, so smm bufs=3 still suffices).
  - Block evacuations split ACT/DVE and acc pool shrunk to 2 bufs.
  - qk(j1) filler split into q/k halves for finer interleave.

Math notes (exact transformations vs the reference):
  - bk dropped: scores[i,j] += q_i . bk is constant in j -> softmax invariant.
  - bv added host-side: softmax rows sum to 1 -> probs @ (1 x bv) = bv.
  - host divides ctxT rows 0..63 by row 64 (softmax denominator) + transposes.
  - additive mask folded multiplicatively via v_ext (65th col = exp(mask)).
"""

import numpy as np
import ml_dtypes
from contextlib import ExitStack
BF = ml_dtypes.bfloat16

B, S, D, H = 2, 2048, 1024, 16
HD = 64
N_CORES = 8
HPC = 4            # heads per core
CW = HPC * HD      # 256 output cols per core
KI = D // 128      # 8 contraction chunks
NSC = S // 128     # 16 k chunks of 128
NQH = 2            # q halves per head
NWARM = 10         # PE warm-up matmuls
USE_DVE_EXP = True

# deg-4 minimax for exp(w), w in [-0.58, 0.58], constrained a0=a1=1:
# exp(w) ~= (1+w) + w^2*((c2 + c3*w) + c4*w^2); rel err 2.0e-4, x16 -> 3.2e-3
EXP_C2 = 0.5007046190058639
EXP_C3 = 0.16849555077816972
EXP_C4 = 0.03865395001612302

_NC_CACHE = []
_DVE_OPS = []


def _register_dve_exp():
    """Register EXP_P4_ANT + POW16_ANT via the documented dve_ops extension
    point (runtime append; the RO install prevents editing dve_ops.py)."""
    if _DVE_OPS:
        return _DVE_OPS
    from concourse import dve_ops
    from concourse.dve_spec import Spec, Src0, One, C0, C1, C2, lower
    from concourse.dve_uop import DveOpSpec
    from concourse.dve_table_gen import dve_ver_for

    ver = dve_ver_for("TRN2")

    w = Src0
    w2 = w * w
    poly = (One + w) + ((C0 + C1 * w) + C2 * w2) * w2
    x = Src0
    x2 = x * x
    x4 = x2 * x2
    x8 = x4 * x4
    specs = [
        ("EXP_P4_ANT", Spec(body=poly)),
        ("POW16_ANT", Spec(body=x8 * x8)),
    ]
    ops = []
    for name, spec in specs:
        existing = [o for o in dve_ops.OPS if o.name == name]
        if existing:
            ops.append(existing[0])
            continue
        row = dve_ops._CUSTOM_DVE_ROW_BASE + len(dve_ops.OPS)
        sha = DveOpSpec(name=name, opcode=row, uops=lower(spec, ver=ver),
                        rd1_en=False).sha(ver)
        op = dve_ops.DveOp(name, spec, subdim=False, uops_sha={ver: sha})
        dve_ops.OPS.append(op)
        dve_ops._SUB_OPCODE_FOR_NAME[name] = row
        ops.append(op)
    _DVE_OPS.extend(ops)
    return ops


def _build_nc():
    import concourse.bacc as bacc
    import concourse.mybir as mybir
    import concourse.tile as tile
    from concourse import masks

    exp_p4, pow16 = _register_dve_exp()

    F32 = mybir.dt.float32
    F32R = mybir.dt.float32r
    BF16 = mybir.dt.bfloat16
    AF = mybir.ActivationFunctionType
    ALU = mybir.AluOpType

    nc = bacc.Bacc("TRN2", target_bir_lowering=False, debug=False)

    hsT_d = nc.dram_tensor("hsT", [D, S], BF16, kind="ExternalInput")
    wq_d = nc.dram_tensor("wq", [D, CW], BF16, kind="ExternalInput")
    wk_d = nc.dram_tensor("wk", [D, CW], BF16, kind="ExternalInput")
    wv_d = nc.dram_tensor("wv", [D, CW], BF16, kind="ExternalInput")
    bq_d = nc.dram_tensor("bq", [CW], F32, kind="ExternalInput")
    mask_d = nc.dram_tensor("mask", [S], F32, kind="ExternalInput")
    # ctxT + denominator row per head: rows 65h..65h+63 = ctx.T, 65h+64 = den
    out_d = nc.dram_tensor("out", [HPC * (HD + 1), S], F32, kind="ExternalOutput")

    hsT_r = hsT_d.rearrange("(ki p) s -> p ki s", p=128)
    wq_r = wq_d.rearrange("(ki p) n -> p ki n", p=128)
    wk_r = wk_d.rearrange("(ki p) n -> p ki n", p=128)
    wv_r = wv_d.rearrange("(ki p) n -> p ki n", p=128)
    bq_r = bq_d.rearrange("(j p) -> p j", p=128)
    mask_r = mask_d.rearrange("(sc p) -> p sc", p=128)

    with tile.TileContext(nc) as tc, ExitStack() as ctx:
        const = ctx.enter_context(tc.tile_pool(name="const", bufs=1))
        load = ctx.enter_context(tc.tile_pool(name="load", bufs=1))
        acts = ctx.enter_context(tc.tile_pool(name="acts", bufs=1))
        ptp = ctx.enter_context(tc.tile_pool(name="ptp", bufs=6))
        outp = ctx.enter_context(tc.tile_pool(name="outp", bufs=4))
        # PSUM: smm "mm" 3x[128,1024] = 6 banks; ctxa "acc" 2x[128,512] = 2.
        smm = ctx.enter_context(tc.tile_pool(name="smm", bufs=3, space="PSUM"))
        ctxa = ctx.enter_context(tc.tile_pool(name="ctxa", bufs=2, space="PSUM"))

        # ---- constants ----
        warmsrc = const.tile([128, 128], F32)
        nc.vector.memset(warmsrc[:], 0.0)
        bq_t = const.tile([128, 2], F32)
        nc.sync.dma_start(bq_t[:], bq_r)
        maskr = const.tile([128, 16], F32)
        nc.sync.dma_start(maskr[:], mask_r)
        emt = const.tile([128, 16], F32)
        nc.scalar.activation(emt[:], maskr[:], AF.Exp)

        # ---- PE warm-up: fp32 identity matmuls while DMAs land (HAM) ----
        warm = smm.tile([128, 1024], F32, tag="mm", name="warm")
        for i in range(NWARM):
            nc.tensor.matmul(warm[:, 0:128], warmsrc[:], warmsrc[:],
                             start=True, stop=True)

        # ---- loads: critical-first on HWDGE(sync); bulk on SWDGE(gpsimd) ----
        hsT_t = load.tile([128, KI, S], BF16)
        wq_t = load.tile([128, KI, CW], BF16)
        wk_t = load.tile([128, KI, CW], BF16)
        wv_t = load.tile([128, KI, CW], BF16)
        for ki in range(KI):
            nc.sync.dma_start(wq_t[:, ki, 0:128], wq_r[:, ki, 0:128])
            nc.sync.dma_start(hsT_t[:, ki, 0:512], hsT_r[:, ki, 0:512])
        for ki in range(KI):
            nc.sync.dma_start(wk_t[:, ki, 0:128], wk_r[:, ki, 0:128])
        for n in range(1, 4):
            for ki in range(KI):
                nc.sync.dma_start(hsT_t[:, ki, 512 * n:512 * (n + 1)],
                                  hsT_r[:, ki, 512 * n:512 * (n + 1)])
        for ki in range(KI):
            nc.gpsimd.dma_start(wq_t[:, ki, 128:256], wq_r[:, ki, 128:256])
        for ki in range(KI):
            nc.gpsimd.dma_start(wk_t[:, ki, 128:256], wk_r[:, ki, 128:256])
        for ki in range(KI):
            nc.gpsimd.dma_start(wv_t[:, ki, :], wv_r[:, ki, :])

        # ---- persistent activations ----
        qT_t = acts.tile([128, 2, S], BF16)
        kT_t = acts.tile([128, 2, S], BF16)
        v_ext = acts.tile([128, NSC, HPC, HD + 1], BF16)
        for h in range(HPC):
            nc.vector.tensor_copy(v_ext[:, :, h, HD], emt[:, :])

        # ---- Q^T (pre-scaled by 1/128) and K^T (zero-padded) ----
        def emit_qk_q(j, n, ps):
            for ki in range(KI):
                nc.tensor.matmul(
                    ps[:, 0:512], wq_t[:, ki, 128 * j:128 * (j + 1)],
                    hsT_t[:, ki, 512 * n:512 * (n + 1)],
                    start=(ki == 0), stop=(ki == KI - 1))
            nc.vector.tensor_scalar(
                qT_t[:, j, 512 * n:512 * (n + 1)], ps[:, 0:512],
                bq_t[:, j:j + 1], 1.0 / 128.0, ALU.add, ALU.mult)

        def emit_qk_k(j, n, ps):
            for ki in range(KI):
                nc.tensor.matmul(
                    ps[:, 512:1024], wk_t[:, ki, 128 * j:128 * (j + 1)],
                    hsT_t[:, ki, 512 * n:512 * (n + 1)],
                    start=(ki == 0), stop=(ki == KI - 1))
            nc.vector.tensor_copy(kT_t[:, j, 512 * n:512 * (n + 1)],
                                  ps[:, 512:1024])

        def emit_qk_n(j, n):
            ps = smm.tile([128, 1024], F32, tag="mm", name=f"qk{j}_{n}")
            emit_qk_q(j, n, ps)
            emit_qk_k(j, n, ps)

        # ---- V chunk sc: [128, 256] -> v_ext[:, sc, :, 0:64] * exp(mask) ----
        def emit_v(sc):
            ps4 = smm.tile([128, 16, 64], F32, tag="mm", name=f"v{sc}")
            for ki in range(KI):
                nc.tensor.matmul(
                    ps4[:, 0:4, :], hsT_t[:, ki, 128 * sc:128 * (sc + 1)],
                    wv_t[:, ki, :], start=(ki == 0), stop=(ki == KI - 1))
            nc.vector.tensor_scalar_mul(
                v_ext[:, sc, :, 0:HD], ps4[:, 0:4, :], emt[:, sc:sc + 1])

        # ---- attention pipeline ----
        # step s = (h, qh, kc). DVE handles exp when kc % 4 == 1 (never the
        # start/stop kc of an accumulation block), ACT otherwise.
        steps = [(j, qc, kc)
                 for j in range(2) for qc in range(4) for kc in range(NSC)]
        nsteps = len(steps)

        def is_dve(s):
            return USE_DVE_EXP and steps[s][2] in (1, 4, 7, 10, 13)

        # filler PE work interleaved into early steps (keyed by sc-emission idx)
        filler = {}
        for kc in range(NSC):          # V chunks during block 0, even slots
            filler.setdefault(kc + (kc % 2), []).append(("v", kc))
        for n in range(4):             # qk j=1 halves during (h0, qh1)
            filler.setdefault(NSC + 4 * n, []).append(("qkq", n))
            filler.setdefault(NSC + 4 * n + 2, []).append(("qkk", n))

        acc_tiles = {}
        pt_tiles = {}
        qk1_ps = {}

        def emit_scores(s):
            j, qc, kc = steps[s]
            ps = smm.tile([128, 1024], F32, tag="mm", name=f"sc{s}")
            qs = slice(512 * qc, 512 * (qc + 1))
            for par in range(2):
                pr = slice(64 * par, 64 * (par + 1))
                nc.tensor.matmul(
                    ps[:, 512 * par:512 * (par + 1)],
                    kT_t[pr, j, 128 * kc:128 * (kc + 1)],
                    qT_t[pr, j, qs],
                    start=True, stop=True, tile_position=(64 * par, 0))
            pt = ptp.tile([128, 1024], BF16, tag="pt", name=f"pt{s}")
            if is_dve(s):
                tmp = ptp.tile([128, 1024], F32, tag="ptm", bufs=2,
                               name=f"tm{s}")
                nc.vector._custom_dve(exp_p4, out=tmp[:], in0=ps[:],
                                      s0=EXP_C2, s1=EXP_C3, imm2=EXP_C4)
                nc.vector._custom_dve(pow16, out=pt[:], in0=tmp[:])
            else:
                nc.scalar.activation(pt[:], ps[:], AF.Exp, scale=16.0)
            pt_tiles[s] = pt

        def emit_ctx(s):
            j, qc, kc = steps[s]
            if kc == 0:
                acc_tiles[s] = [
                    ctxa.tile([128, 512], F32, tag="acc", name=f"acc{s}_{p}")
                    for p in range(2)]
            accs = acc_tiles[s - kc]
            pt = pt_tiles.pop(s)
            for par in range(2):
                nc.tensor.matmul(
                    accs[par][0:65, :], v_ext[:, kc, 2 * j + par, :],
                    pt[:, 512 * par:512 * (par + 1)],
                    start=(kc == 0), stop=(kc == NSC - 1))
            if kc == NSC - 1:
                for par in range(2):
                    h = 2 * j + par
                    cts = outp.tile([65, 512], F32, tag="cts", name=f"c{s}_{par}")
                    if par == 0:
                        nc.scalar.copy(cts[:], accs[par][0:65, :])
                    else:
                        nc.vector.tensor_copy(cts[:], accs[par][0:65, :])
                    nc.sync.dma_start(
                        out_d[65 * h:65 * (h + 1),
                              512 * qc:512 * (qc + 1)], cts[:])
                del acc_tiles[s - kc]

        for n in range(4):
            emit_qk_n(0, n)
        for i in range(nsteps):
            for kind, arg in filler.get(i, []):
                if kind == "v":
                    emit_v(arg)
                elif kind == "qkq":
                    ps = smm.tile([128, 1024], F32, tag="mm", name=f"q1_{arg}")
                    qk1_ps[arg] = ps
                    emit_qk_q(1, arg, ps)
                else:
                    emit_qk_k(1, arg, qk1_ps.pop(arg))
            emit_scores(i)
            if i % 2 == 1 and i >= 5:
                emit_ctx(i - 5)
                emit_ctx(i - 4)
        for s in range(nsteps - 4, nsteps):
            emit_ctx(s)

    nc.finalize()
    return nc


def _get_nc():
    if not _NC_CACHE:
        _NC_CACHE.append(_build_nc())
    return _NC_CACHE[0]


def _shard_inputs(hidden_states, attention_mask, Wq, bq, Wk, Wv):
    hsT = [np.ascontiguousarray(hidden_states[b].T) for b in range(B)]
    in_maps = []
    for c in range(N_CORES):
        b, g = divmod(c, N_CORES // B)
        cs = slice(CW * g, CW * (g + 1))
        in_maps.append({
            "hsT": hsT[b].astype(BF),
            "wq": np.ascontiguousarray(Wq[:, cs]).astype(BF),
            "wk": np.ascontiguousarray(Wk[:, cs]).astype(BF),
            "wv": np.ascontiguousarray(Wv[:, cs]).astype(BF),
            "bq": np.ascontiguousarray(bq[cs]),
            "mask": np.ascontiguousarray(attention_mask[b, 0, 0, :]),
        })
    return in_maps


def kernel(hidden_states, attention_mask, Wq, bq, Wk, bk, Wv, bv):
    from concourse.bass_utils import run_bass_kernel_spmd

    hidden_states = np.asarray(hidden_states, dtype=np.float32)
    attention_mask = np.asarray(attention_mask, dtype=np.float32)
    Wq = np.asarray(Wq, dtype=np.float32)
    Wk = np.asarray(Wk, dtype=np.float32)
    Wv = np.asarray(Wv, dtype=np.float32)
    bq = np.asarray(bq, dtype=np.float32)
    bv = np.asarray(bv, dtype=np.float32)

    in_maps = _shard_inputs(hidden_states, attention_mask, Wq, bq, Wk, Wv)
    res = run_bass_kernel_spmd(_get_nc(), in_maps, core_ids=list(range(N_CORES)))

    out = np.empty((B, S, D), dtype=np.float32)
    for c in range(N_CORES):
        b, g = divmod(c, N_CORES // B)
        o = res.results[c]["out"].astype(np.float64)  # [260, 2048]
        for h in range(HPC):
            ctx = o[65 * h:65 * h + 64, :] / o[65 * h + 64, :]
            out[b, :, CW * g + HD * h:CW * g + HD * (h + 1)] = ctx.T
    out += bv  # exact: probs rows sum to 1
    return out
